# revision 7
# baseline (speedup 1.0000x reference)
# Trainium2 Bass kernel for CustomFullyConnectedLayer:
#   y = x @ W.T,  W[(c+i)%N, c] += V[i, c] for i in diag_pos  (banded weight)
# Strategy: data-parallel over batch across 8 cores; compute y.T directly
# from host-pre-transposed x so no on-chip transposes are needed.
#   y.T[r, b] = sum_i V[i, r-i] * x.T[(r-i)%N, b]
# Per 128-row output tile j (r = 128j+q):
#   psum[q, b] = sum_p A_j[p, q] xT[128j+p, b] + sum_pp B_j[pp, q] xT[128(j-1)+96+pp, b]
# where A_j is the in-tile band (i <= q) and B_j the 32-row wrap band from
# the previous feature tile (i > q). Both are built on the host (free).
import os
import sys

import numpy as np

if "/opt/trn_rl_repo" not in sys.path:
    sys.path.insert(0, "/opt/trn_rl_repo")

import ml_dtypes

BATCH = 8192
N = 3072
NCORES = 8
BC = BATCH // NCORES          # 1024 batch cols per core
NJ = N // 128                 # 24 feature/output tiles
HB = BC // 2                  # psum chunk (one bank of fp32)

_CACHE = {}
LAST_RESULTS = None


def _build_program():
    import concourse.mybir as mybir
    import concourse.tile as tile
    from concourse import bacc

    cdt = mybir.dt.bfloat16
    f32 = mybir.dt.float32

    nc = bacc.Bacc("TRN2", target_bir_lowering=False, debug=False)
    xT = nc.dram_tensor("xT", [N, BC], cdt, kind="ExternalInput")
    wa = nc.dram_tensor("wa", [128, NJ, 128], cdt, kind="ExternalInput")
    wb = nc.dram_tensor("wb", [64, NJ, 128], cdt, kind="ExternalInput")
    yT = nc.dram_tensor("yT", [N, BC], cdt, kind="ExternalOutput")

    with tile.TileContext(nc) as tc:
        with (
            tc.tile_pool(name="consts", bufs=1) as consts,
            tc.tile_pool(name="yout", bufs=3) as yout,
            tc.tile_pool(name="ps", bufs=6, space="PSUM") as psp,
        ):
            a_sb = consts.tile([128, NJ, 128], cdt)
            b_sb = consts.tile([128, NJ, 128], cdt)  # rows 64:128 hold B
            x_sb = consts.tile([128, NJ, BC], cdt)

            # x tile 23 first: j=0 needs it for the wrap band.
            nc.sync.dma_start(
                out=x_sb[:, NJ - 1, :], in_=xT[128 * (NJ - 1):, :]
            )
            nc.scalar.dma_start(out=a_sb, in_=wa[:, :, :])
            nc.scalar.dma_start(out=b_sb[64:128, :, :], in_=wb[:, :, :])
            for j in range(NJ - 1):
                nc.sync.dma_start(
                    out=x_sb[:, j, :], in_=xT[128 * j: 128 * (j + 1), :]
                )

            # PE warm-up on zeros: opens the HAM clock gate (0.65/1.2 ->
            # 2.4 GHz) while the first DMAs land.
            wsrc = consts.tile([128, 128], cdt)
            nc.vector.memset(wsrc, 0.0)
            wps = psp.tile([128, HB], f32, tag="ps")
            for _ in range(28):
                nc.tensor.matmul(
                    wps[:, :128], lhsT=wsrc, rhs=wsrc, start=True, stop=True
                )

            for j in range(NJ):
                jm1 = (j - 1) % NJ
                ps0 = psp.tile([128, HB], f32, tag="ps")
                ps1 = psp.tile([128, HB], f32, tag="ps")
                nc.tensor.matmul(
                    ps0, lhsT=a_sb[:, j, :], rhs=x_sb[:, j, 0:HB],
                    start=True, stop=False,
                )
                nc.tensor.matmul(
                    ps1, lhsT=a_sb[:, j, :], rhs=x_sb[:, j, HB:BC],
                    start=True, stop=False, skip_group_check=True,
                )
                nc.tensor.matmul(
                    ps0, lhsT=b_sb[64:128, j, :], rhs=x_sb[64:128, jm1, 0:HB],
                    start=False, stop=True, skip_group_check=True,
                )
                nc.tensor.matmul(
                    ps1, lhsT=b_sb[64:128, j, :], rhs=x_sb[64:128, jm1, HB:BC],
                    start=False, stop=True, skip_group_check=True,
                )
                y_sb = yout.tile([128, BC], cdt)
                nc.scalar.copy(out=y_sb[:, 0:HB], in_=ps0)
                nc.vector.tensor_copy(out=y_sb[:, HB:BC], in_=ps1)
                rows = slice(128 * j, 128 * (j + 1))
                if j == NJ - 1:
                    # split the last store so the drain overlaps the copies
                    nc.gpsimd.dma_start(out=yT[rows, 0:HB], in_=y_sb[:, 0:HB])
                    nc.gpsimd.dma_start(out=yT[rows, HB:BC], in_=y_sb[:, HB:BC])
                else:
                    nc.gpsimd.dma_start(out=yT[rows, :], in_=y_sb)

    nc.compile()
    return nc


def _host_prep(x, V, diag_pos):
    bf16 = ml_dtypes.bfloat16
    diag = np.asarray(diag_pos).astype(np.int64) % N
    if diag.size and int(diag.max()) > 29:
        raise ValueError(
            f"band kernel supports diag offsets <= 29, got {int(diag.max())}"
        )
    V32 = np.asarray(V, dtype=np.float32)

    A = np.zeros((NJ, 128, 128), np.float32)
    B = np.zeros((NJ, 64, 128), np.float32)
    jj = np.arange(NJ)[:, None]
    for i in diag:
        i = int(i)
        p = np.arange(128 - i)[None, :]
        A[jj, p, p + i] += V32[i, (128 * jj + p) % N]
        if i > 0:
            pp = np.arange(64 - i, 64)[None, :]
            B[jj, pp, pp + i - 64] += V32[i, (128 * (jj - 1) + 64 + pp) % N]
    wa = np.ascontiguousarray(A.transpose(1, 0, 2)).astype(bf16)
    wb = np.ascontiguousarray(B.transpose(1, 0, 2)).astype(bf16)

    xb = np.ascontiguousarray(np.asarray(x, dtype=np.float32)).astype(bf16)
    xb = xb.view(np.uint16)
    xTs = [
        np.ascontiguousarray(xb[k * BC:(k + 1) * BC, :].T).view(bf16)
        for k in range(NCORES)
    ]
    return xTs, wa, wb


def kernel(x, V, diag_pos):
    global LAST_RESULTS
    from concourse.bass_utils import run_bass_kernel_spmd

    if "prog" not in _CACHE:
        _CACHE["prog"] = _build_program()
    nc = _CACHE["prog"]

    xTs, wa, wb = _host_prep(x, V, diag_pos)
    in_maps = [
        {"xT": xTs[k], "wa": wa, "wb": wb} for k in range(NCORES)
    ]
    res = run_bass_kernel_spmd(nc, in_maps, core_ids=list(range(NCORES)))
    LAST_RESULTS = res
    out = np.empty((BATCH, N), np.float32)
    for k in range(NCORES):
        yTk = np.asarray(res.results[k]["yT"]).astype(np.float32)
        out[k * BC:(k + 1) * BC, :] = yTk.T
    return out


# revision 9
# speedup vs baseline: 1.0526x; 1.0526x over previous
# Trainium2 Bass kernel for CustomFullyConnectedLayer:
#   y = x @ W.T,  W[(c+i)%N, c] += V[i, c] for i in diag_pos  (banded weight)
# Strategy: data-parallel over batch across 8 cores; compute y.T directly
# from host-pre-transposed x so no on-chip transposes are needed.
#   y.T[r, b] = sum_i V[i, r-i] * x.T[(r-i)%N, b]
# Per 128-row output tile j (r = 128j+q):
#   psum[q, b] = sum_p A_j[p, q] xT[128j+p, b] + sum_pp B_j[pp, q] xT[128(j-1)+96+pp, b]
# where A_j is the in-tile band (i <= q) and B_j the 32-row wrap band from
# the previous feature tile (i > q). Both are built on the host (free).
import os
import sys

import numpy as np

if "/opt/trn_rl_repo" not in sys.path:
    sys.path.insert(0, "/opt/trn_rl_repo")

import ml_dtypes

BATCH = 8192
N = 3072
NCORES = 8
BC = BATCH // NCORES          # 1024 batch cols per core
NJ = N // 128                 # 24 feature/output tiles
HB = BC // 2                  # psum chunk (one bank of fp32)

_CACHE = {}
LAST_RESULTS = None


def _build_program():
    import concourse.mybir as mybir
    import concourse.tile as tile
    from concourse import bacc

    cdt = mybir.dt.bfloat16
    f32 = mybir.dt.float32

    nc = bacc.Bacc("TRN2", target_bir_lowering=False, debug=False)
    xT = nc.dram_tensor("xT", [N, BC], cdt, kind="ExternalInput")
    wa = nc.dram_tensor("wa", [128, NJ, 128], cdt, kind="ExternalInput")
    wb = nc.dram_tensor("wb", [64, NJ, 128], cdt, kind="ExternalInput")
    yT = nc.dram_tensor("yT", [N, BC], cdt, kind="ExternalOutput")

    with tile.TileContext(nc) as tc:
        with (
            tc.tile_pool(name="consts", bufs=1) as consts,
            tc.tile_pool(name="yout", bufs=8) as yout,
            tc.tile_pool(name="ps", bufs=8, space="PSUM") as psp,
        ):
            a_sb = consts.tile([128, NJ, 128], cdt)
            b_sb = consts.tile([128, NJ, 128], cdt)  # rows 64:128 hold B
            x_sb = consts.tile([128, NJ, BC], cdt)

            # x tile 23 first: j=0 needs it for the wrap band.
            nc.sync.dma_start(
                out=x_sb[:, NJ - 1, :], in_=xT[128 * (NJ - 1):, :]
            )
            nc.scalar.dma_start(out=a_sb, in_=wa[:, :, :])
            nc.scalar.dma_start(out=b_sb[64:128, :, :], in_=wb[:, :, :])
            for j in range(NJ - 1):
                nc.sync.dma_start(
                    out=x_sb[:, j, :], in_=xT[128 * j: 128 * (j + 1), :]
                )

            # PE warm-up on zeros: opens the HAM clock gate (0.65/1.2 ->
            # 2.4 GHz) while the first DMAs land.
            wsrc = consts.tile([128, 128], cdt)
            nc.vector.memset(wsrc, 0.0)
            wps = psp.tile([128, HB], f32, tag="ps")
            for _ in range(14):
                nc.tensor.matmul(
                    wps[:, :128], lhsT=wsrc, rhs=wsrc, start=True, stop=True
                )

            for j in range(NJ):
                jm1 = (j - 1) % NJ
                ps0 = psp.tile([128, HB], f32, tag="ps")
                ps1 = psp.tile([128, HB], f32, tag="ps")
                nc.tensor.matmul(
                    ps0, lhsT=a_sb[:, j, :], rhs=x_sb[:, j, 0:HB],
                    start=True, stop=False,
                )
                nc.tensor.matmul(
                    ps1, lhsT=a_sb[:, j, :], rhs=x_sb[:, j, HB:BC],
                    start=True, stop=False, skip_group_check=True,
                )
                nc.tensor.matmul(
                    ps0, lhsT=b_sb[64:128, j, :], rhs=x_sb[64:128, jm1, 0:HB],
                    start=False, stop=True, skip_group_check=True,
                )
                nc.tensor.matmul(
                    ps1, lhsT=b_sb[64:128, j, :], rhs=x_sb[64:128, jm1, HB:BC],
                    start=False, stop=True, skip_group_check=True,
                )
                y_sb = yout.tile([128, BC], cdt)
                nc.scalar.copy(out=y_sb[:, 0:HB], in_=ps0)
                nc.vector.tensor_copy(out=y_sb[:, HB:BC], in_=ps1)
                rows = slice(128 * j, 128 * (j + 1))
                if j == NJ - 1:
                    # split the last store so the drain overlaps the copies
                    nc.gpsimd.dma_start(out=yT[rows, 0:HB], in_=y_sb[:, 0:HB])
                    nc.gpsimd.dma_start(out=yT[rows, HB:BC], in_=y_sb[:, HB:BC])
                else:
                    nc.gpsimd.dma_start(out=yT[rows, :], in_=y_sb)

    nc.compile()
    return nc


def _host_prep(x, V, diag_pos):
    bf16 = ml_dtypes.bfloat16
    diag = np.asarray(diag_pos).astype(np.int64) % N
    if diag.size and int(diag.max()) > 29:
        raise ValueError(
            f"band kernel supports diag offsets <= 29, got {int(diag.max())}"
        )
    V32 = np.asarray(V, dtype=np.float32)

    A = np.zeros((NJ, 128, 128), np.float32)
    B = np.zeros((NJ, 64, 128), np.float32)
    jj = np.arange(NJ)[:, None]
    for i in diag:
        i = int(i)
        p = np.arange(128 - i)[None, :]
        A[jj, p, p + i] += V32[i, (128 * jj + p) % N]
        if i > 0:
            pp = np.arange(64 - i, 64)[None, :]
            B[jj, pp, pp + i - 64] += V32[i, (128 * (jj - 1) + 64 + pp) % N]
    wa = np.ascontiguousarray(A.transpose(1, 0, 2)).astype(bf16)
    wb = np.ascontiguousarray(B.transpose(1, 0, 2)).astype(bf16)

    xb = np.ascontiguousarray(np.asarray(x, dtype=np.float32)).astype(bf16)
    xb = xb.view(np.uint16)
    xTs = [
        np.ascontiguousarray(xb[k * BC:(k + 1) * BC, :].T).view(bf16)
        for k in range(NCORES)
    ]
    return xTs, wa, wb


def kernel(x, V, diag_pos):
    global LAST_RESULTS
    from concourse.bass_utils import run_bass_kernel_spmd

    if "prog" not in _CACHE:
        _CACHE["prog"] = _build_program()
    nc = _CACHE["prog"]

    xTs, wa, wb = _host_prep(x, V, diag_pos)
    in_maps = [
        {"xT": xTs[k], "wa": wa, "wb": wb} for k in range(NCORES)
    ]
    res = run_bass_kernel_spmd(nc, in_maps, core_ids=list(range(NCORES)))
    LAST_RESULTS = res
    out = np.empty((BATCH, N), np.float32)
    for k in range(NCORES):
        yTk = np.asarray(res.results[k]["yT"]).astype(np.float32)
        out[k * BC:(k + 1) * BC, :] = yTk.T
    return out
